# revision 28
# baseline (speedup 1.0000x reference)
"""Trainium2 Bass kernel for Masked_Actor_Net_PNAConv (3x PNAConv + gated masked softmax head).

Sharding: data-parallel by graph across 8 NeuronCores (8 graphs / 2048 nodes /
16384 edges per core). Weights replicated. BatchNorm batch stats are
all-reduced across cores (one [128, 2*Fo] f32 AllReduce per conv layer).

Device-side structure (per core, per layer):
  - h kept feature-major in SBUF: hT [128, F, 2048] bf16
  - A = h @ Wm_src computed node-major on PE -> a_sb (SBUF only)
  - per-graph one-hot src matrices S (built once from src indices via K=1
    broadcast matmuls + is_equal compares) turn the edge gather into PE
    matmuls: msgT chunk = A_g^T @ S_half0 + A_g^T @ S_half1 + Wmc^T @ esT,
    all accumulated in one PSUM tile [128, 1024]
  - aggregations read PSUM directly: max via windowed tensor_reduce
    ([128, 128dst, 8edge] -> [128, 128]), sum-of-squares via Square-evict
    (scalar/vector alternating) + windowed add-reduce; sum via PE matmul
    with host-built adjacency count matrices + es-sum term
  - U matmuls run per graph PAIR (512-col rhs) with folded weights; BN
    folded into the mixing Linear after the stats AllReduce
  - head: exp-softmax without max subtraction (values bounded), mask folded
    multiplicatively, partition sums via ones-matmuls (no gpsimd)
"""
import sys
sys.path.insert(0, '/opt/trn_rl_repo')
import contextlib
import numpy as np
import ml_dtypes

import concourse.bacc as bacc
import concourse.mybir as mybir
import concourse.bass_isa as bass_isa
from concourse import tile
from concourse.bass_utils import run_bass_kernel_spmd

BF = mybir.dt.bfloat16
F32 = mybir.dt.float32
AL = mybir.AluOpType
AF = mybir.ActivationFunctionType
AX = mybir.AxisListType

B, NN, DEG = 64, 256, 8
N, E = B * NN, B * NN * DEG
IN_N, IN_E = 128, 16
TP = 192
H1 = 384
NCORES = 8
G = B // NCORES        # 8 graphs per core
NC = G * NN            # 2048 nodes per core
EC = NC * DEG          # 16384 edges per core

CIN = [IN_N, H1 + 32, H1]                 # 128, 416, 384
COUT = [H1, H1, TP]                       # 384, 384, 192
NF = [(c + 127) // 128 for c in CIN]      # 1, 4, 3
CINP = [128 * f for f in NF]              # 128, 512, 384
NFO = [(c + 127) // 128 for c in COUT]    # 3, 3, 2
CSZ = [[min(128, CIN[k] - 128 * i) for i in range(NF[k])] for k in range(3)]
MSZ = [[min(128, COUT[k] - 128 * i) for i in range(NFO[k])] for k in range(3)]
RSQ8 = float(np.float32(1.0 / np.sqrt(8.0)))   # mean-section pre-scale

_BUILT = {}


def _bf(x):
    return np.ascontiguousarray(np.asarray(x, np.float32).astype(ml_dtypes.bfloat16))


def _f32(x):
    return np.ascontiguousarray(np.asarray(x, np.float32))


# ---------------------------------------------------------------------------
# device kernel (SPMD, identical program on all 8 cores)
# ---------------------------------------------------------------------------

def build_nc():
    import os as _os
    STAGE = int(_os.environ.get("KERN_STAGE", "7"))
    nc = bacc.Bacc(None, target_bir_lowering=False, debug=True)

    def par(name, shape, dt, out=False):
        return nc.declare_dram_parameter(name, list(shape), dt, isOutput=out)

    p_nsT = par("nsT", [128, 2048], BF)
    p_dmT = par("dmT", [128, 2 * 2048], BF)
    p_esT = par("esT", [16, EC], BF)
    p_esagg = par("esagg", [16, 2048], BF)
    p_src = par("srcbf", [1, EC], BF)
    p_iota2 = par("iota2", [128, 2], F32)
    p_ones = par("onesbf", [1, 128], BF)
    p_hoc = par("honesc", [128, 1], F32)
    p_hor = par("honesr", [1, 128], F32)
    p_madj = par("madj", [128, G * 2 * 256], BF)
    p_mask = par("maskT", [128, 16 * 192], BF)
    p_wma = [par(f"wma{k}", [128, NF[k] * CINP[k]], BF) for k in range(3)]
    p_wmc = [par(f"wmc{k}", [16, CINP[k]], BF) for k in range(3)]
    p_wu = [par(f"wu{k}", [128, 4 * NF[k] * COUT[k]], BF) for k in range(3)]
    p_wx = [par(f"wx{k}", [128, NFO[k] * COUT[k]], BF) for k in range(3)]
    p_gam = [par(f"gam{k}", [128, NFO[k]], F32) for k in range(3)]
    p_bh = [par(f"bh{k}", [128, NFO[k]], F32) for k in range(2)]
    p_w12 = par("w12", [128, 2 * 32], BF)
    p_b12 = par("b12", [32, 1], F32)
    p_w3 = par("w3", [128, 2 * 64], BF)
    p_b3 = par("b3", [64, 1], F32)
    p_w4 = par("w4", [64, 256], BF)
    p_b4 = par("b4", [128, 2], F32)
    p_out = par("out", [128, 16 * 192], F32, out=True)
    DBG = _os.environ.get("KERN_DEBUG")
    if DBG:
        p_dbg_a = par("dbg_a", [128, 16 * 128], BF, out=True)
        p_dbg_agg = par("dbg_agg", [128, 3 * 512], BF, out=True)   # pmax|pmnx|pstd pair0 L0 f0
        p_dbg_q = par("dbg_q", [128, 512], F32, out=True)
        p_dbg_u = par("dbg_u", [128, 3 * 2048], BF, out=True)
        p_dbg_s = par("dbg_s", [128, 2 * 2048], BF, out=True)
        p_dbg_h1 = par("dbg_h1", [128, 4 * 2048], BF, out=True)
        p_dbg_h2 = par("dbg_h2", [128, 3 * 2048], BF, out=True)
        p_dbg_h3 = par("dbg_h3", [128, 16 * 192], BF, out=True)
        p_dbg_nmx = par("dbg_nmx", [128, 16], BF, out=True)
        p_dbg_hd = par("dbg_hd", [128, 32], F32, out=True)  # gn|gsum|ginv
        p_dbg_bn = par("dbg_bn", [128, 32], F32, out=True)  # ccs|ccr|mu|sc|uaccS|uaccQ (L0)

    with tile.TileContext(nc) as tc:
        with contextlib.ExitStack() as ctx:
            stat = ctx.enter_context(tc.tile_pool(name="stat", bufs=1))
            sall = ctx.enter_context(tc.tile_pool(name="sall", bufs=1))   # S / head scratch
            abuf = ctx.enter_context(tc.tile_pool(name="abuf", bufs=1))   # a_sb / dmT
            sqp = ctx.enter_context(tc.tile_pool(name="sqp", bufs=2))     # msgsq slots
            aggp = ctx.enter_context(tc.tile_pool(name="aggp", bufs=1))   # pair aggregates
            wupool = ctx.enter_context(tc.tile_pool(name="wupool", bufs=1))
            espool = ctx.enter_context(tc.tile_pool(name="espool", bufs=2))
            dpool = ctx.enter_context(tc.tile_pool(name="dpool", bufs=1, space="DRAM"))
            psMsg = ctx.enter_context(tc.tile_pool(name="psMsg", bufs=2, space="PSUM"))
            psAgg = ctx.enter_context(tc.tile_pool(name="psAgg", bufs=1, space="PSUM"))
            psE = ctx.enter_context(tc.tile_pool(name="psE", bufs=2, space="PSUM"))

            def load(shape, dt, src, tag, pool=None):
                t = (pool or stat).tile(list(shape), dt, tag=tag, name=tag)
                nc.sync.dma_start(t[:], src[:])
                return t

            hT = [None, None, None]
            hT[0] = load([128, 1, 2048], BF, p_nsT, "hT0")
            hT[1] = stat.tile([128, 4, 2048], BF, tag="hT1", name="hT1")
            hT[2] = stat.tile([128, 3, 2048], BF, tag="hT2", name="hT2")
            uT = stat.tile([128, 3, 2048], BF, tag="uT")
            esagg = load([16, 2048], BF, p_esagg, "esagg")
            madj = load([128, G, 2, 256], BF, p_madj, "madj")
            iota2 = load([128, 2], F32, p_iota2, "iota2")
            onesbf = load([1, 128], BF, p_ones, "onesbf")
            honesc = load([128, 1], F32, p_hoc, "honesc")
            honesr = load([1, 128], F32, p_hor, "honesr")
            wma = [load([128, NF[k], CINP[k]], BF, p_wma[k], f"wma{k}s") for k in range(3)]
            wmc = [load([16, CINP[k]], BF, p_wmc[k], f"wmc{k}s") for k in range(3)]
            wx = [load([128, NFO[k], COUT[k]], BF, p_wx[k], f"wx{k}s") for k in range(3)]
            gam = [load([128, NFO[k]], F32, p_gam[k], f"gam{k}s") for k in range(3)]
            bh = [load([128, NFO[k]], F32, p_bh[k], f"bh{k}s") for k in range(2)]
            w12 = load([128, 2, 32], BF, p_w12, "w12")
            b12 = load([32, 1], F32, p_b12, "b12")
            w3 = load([128, 2, 64], BF, p_w3, "w3")
            b3 = load([64, 1], F32, p_b3, "b3")
            w4 = load([64, 256], BF, p_w4, "w4")
            b4 = load([128, 2], F32, p_b4, "b4")
            wxs = stat.tile([128, 3, 384], BF, tag="wxs")
            cc_in = [dpool.tile([128, 2 * NFO[k]], F32, tag=f"ccin{k}", name=f"ccin{k}") for k in range(3)]
            cc_out = [dpool.tile([128, 2 * NFO[k]], F32, tag=f"ccout{k}", name=f"ccout{k}") for k in range(3)]

            if DBG:
                nc.vector.memset(hT[1][:, 3, :], 0.0)  # rows 32+ never written; dump reads full tile

            # S one-hot matrices, all graphs: S[p, g, half, e] = (src[e] == p + 128*half)
            S = sall.tile([128, G, 2, 2048], BF, tag="Sall", name="Sall")
            for g in range(G):
                srcg = espool.tile([1, 2048], BF, tag="esg")
                nc.sync.dma_start(srcg[:], p_src[0:1, 2048 * g:2048 * (g + 1)])
                for h2 in range(2):
                    psb = psMsg.tile([128, 1024], F32, tag="psMsg")
                    for e4 in range(2):
                        nc.tensor.matmul(
                            psb[:, 512 * e4:512 * (e4 + 1)], onesbf[0:1, :],
                            srcg[0:1, 1024 * h2 + 512 * e4:1024 * h2 + 512 * (e4 + 1)],
                            start=True, stop=True)
                    for half in range(2):
                        nc.vector.tensor_scalar(
                            S[:, g, half, 1024 * h2:1024 * (h2 + 1)], psb[:, :],
                            iota2[:, half:half + 1], None, AL.is_equal)

            # ---- d2 = dm @ (W1 @ W2) + b12 -> hT[1] chunk 3 rows 0:32 --------
            dmT = abuf.tile([128, 2, 2048], BF, tag="a_sb")
            nc.sync.dma_start(dmT[:].rearrange("p c n -> p (c n)"), p_dmT[:])
            for n4 in range(4):
                ps = psE.tile([128, 512], F32, tag="psE")
                for kc in range(2):
                    nc.tensor.matmul(ps[0:32, :], w12[:, kc, :],
                                     dmT[:, kc, 512 * n4:512 * (n4 + 1)],
                                     start=(kc == 0), stop=(kc == 1))
                nc.scalar.activation(hT[1][0:32, 3, 512 * n4:512 * (n4 + 1)], ps[0:32, :],
                                     AF.Identity, bias=b12[:, 0:1])

            h3 = stat.tile([128, 16, 192], BF, tag="hT0")  # reuses hT0 slot (dead after layer 0)
            c30 = stat.tile([128, 1], F32, tag="c30")
            nc.vector.memset(c30[:], 1e-30)
            c5 = stat.tile([128, 1], F32, tag="c5")
            nc.vector.memset(c5[:], 1e-5)
            uaccS = stat.tile([128, 3, 4], F32, tag="uaccS")
            uaccQ = stat.tile([128, 3, 4], F32, tag="uaccQ")
            sqctr = [0]

            # ---- conv layers -------------------------------------------------
            for k in range(3 if STAGE >= 6 else 1):
                F = NF[k]
                cinp, cout, Fo = CINP[k], COUT[k], NFO[k]
                csz, msz = CSZ[k], MSZ[k]
                h = hT[k]

                wu_k = load([128, 4 * F, cout], BF, p_wu[k], "wu_k", pool=wupool)
                nc.vector.memset(uaccS[:], 0.0)
                nc.vector.memset(uaccQ[:], 0.0)
                if k == 2:
                    nc.vector.memset(uT[64:128, 1, :], 0.0)

                # A = h @ Wma (node-major) -> a_sb
                a_sb = abuf.tile([128, 16, cinp], BF, tag="a_sb")
                for t in range(16):
                    ps = psE.tile([128, 512], F32, tag="psE")
                    for ki in range(F):
                        nc.tensor.matmul(ps[:, 0:cinp],
                                         h[0:csz[ki], ki, 128 * t:128 * (t + 1)],
                                         wma[k][0:csz[ki], ki, :],
                                         start=(ki == 0), stop=(ki == F - 1))
                    nc.scalar.activation(a_sb[:, t, :], ps[:, 0:cinp], AF.Copy, bias=0.0)

                for pr in range(4 if STAGE >= 3 else 0):
                    pmax = aggp.tile([128, F, 512], BF, tag="pmax")
                    qsum = aggp.tile([128, F, 512], F32, tag="qsum")
                    pmnx = aggp.tile([128, F, 512], BF, tag="pmnx")
                    m2 = aggp.tile([128, F, 512], BF, tag="pstd")  # slot shared: m2 dead before pstd written
                    for gs in range(2):
                        g = 2 * pr + gs
                        esg = espool.tile([16, 2048], BF, tag="esg")
                        nc.sync.dma_start(esg[:], p_esT[0:16, 2048 * g:2048 * (g + 1)])
                        for f in range(F):
                            for h2 in range(2):
                                ps = psMsg.tile([128, 1024], F32, tag="psMsg")
                                for e4 in range(2):
                                    sl = ps[:, 512 * e4:512 * (e4 + 1)]
                                    ec = 1024 * h2 + 512 * e4
                                    nc.tensor.matmul(
                                        sl, a_sb[:, 2 * g, 128 * f:128 * (f + 1)],
                                        S[:, g, 0, ec:ec + 512], start=True, stop=False)
                                    nc.tensor.matmul(
                                        sl, a_sb[:, 2 * g + 1, 128 * f:128 * (f + 1)],
                                        S[:, g, 1, ec:ec + 512], start=False, stop=False)
                                    nc.tensor.matmul(
                                        sl, wmc[k][0:16, 128 * f:128 * (f + 1)],
                                        esg[0:16, ec:ec + 512], start=False, stop=True)
                                if STAGE < 4:
                                    continue
                                d0 = 256 * gs + 128 * h2
                                nc.vector.tensor_reduce(
                                    pmax[:, f, d0:d0 + 128],
                                    ps[:].rearrange("p (d j) -> p d j", j=8),
                                    AX.X, AL.max)
                                # msq written j-major so the sum-of-squares tree
                                # uses dense bf16 TT adds (2x DVE mode)
                                msq = sqp.tile([128, 8, 128], BF, tag="msq")
                                nc.scalar.activation(
                                    msq[:].rearrange("p j d -> p d j"),
                                    ps[:].rearrange("p (d j) -> p d j", j=8),
                                    AF.Square)
                                eng = nc.gpsimd if sqctr[0] % 3 == 0 else nc.vector
                                sqctr[0] += 1
                                eng.tensor_tensor(msq[:, 0:4, :], msq[:, 0:4, :], msq[:, 4:8, :], AL.add)
                                eng.tensor_tensor(msq[:, 0:2, :], msq[:, 0:2, :], msq[:, 2:4, :], AL.add)
                                eng.tensor_tensor(qsum[:, f, d0:d0 + 128], msq[:, 0, :], msq[:, 1, :], AL.add)
                        if STAGE < 4:
                            continue
                        # sum aggregation on PE: adjacency matmul + es-sum term
                        psagg = psAgg.tile([128, 1024], F32, tag="psAgg")
                        for f in range(F):
                            sl = psagg[:, 256 * f:256 * (f + 1)]
                            nc.tensor.matmul(sl, a_sb[:, 2 * g, 128 * f:128 * (f + 1)],
                                             madj[:, g, 0, :], start=True, stop=False)
                            nc.tensor.matmul(sl, a_sb[:, 2 * g + 1, 128 * f:128 * (f + 1)],
                                             madj[:, g, 1, :], start=False, stop=False)
                            nc.tensor.matmul(sl, wmc[k][0:16, 128 * f:128 * (f + 1)],
                                             esagg[0:16, 256 * g:256 * (g + 1)],
                                             start=False, stop=True)
                        # pmnx = sum/sqrt(8)  (mean-section weights pre-scaled on host)
                        nc.scalar.activation(
                            pmnx[:, 0:F, 256 * gs:256 * (gs + 1)],
                            psagg[:, 0:256 * F].rearrange("p (f d) -> p f d", f=F),
                            AF.Copy, bias=0.0, scale=RSQ8)
                    if STAGE < 4:
                        continue
                    # stats per pair: var*8 = qsum - pmnx^2 ; pstd = sqrt(var + 1e-30)
                    nc.gpsimd.tensor_tensor(m2[:], pmnx[:], pmnx[:], AL.mult)
                    nc.vector.tensor_tensor(qsum[:], qsum[:], m2[:], AL.subtract)
                    nc.vector.tensor_scalar(qsum[:], qsum[:], 0.0, None, AL.max)
                    pstd = aggp.tile([128, F, 512], BF, tag="pstd")
                    nc.scalar.activation(pstd[:], qsum[:], AF.Sqrt, bias=c30[:, 0:1],
                                         scale=0.125)
                    if DBG and k == 0 and pr == 0:
                        nc.sync.dma_start(p_dbg_agg[:, 0 * 512:1 * 512], pmax[:, 0, :])
                        nc.sync.dma_start(p_dbg_agg[:, 1 * 512:2 * 512], pmnx[:, 0, :])
                        nc.sync.dma_start(p_dbg_agg[:, 2 * 512:3 * 512], pstd[:, 0, :])
                        nc.sync.dma_start(p_dbg_q[:], qsum[:, 0, :])
                    if STAGE < 5:
                        continue
                    # U matmuls: X = [h | mean' | max | std], folded weights, 512-col
                    xs = [None, pmnx, pmax, pstd]
                    for mo in range(Fo):
                        mi = msz[mo]
                        ps = psE.tile([128, 512], F32, tag="psE")
                        nmm = 4 * F
                        i = 0
                        for sect in range(4):
                            for f in range(F):
                                if sect == 0:
                                    rhs = h[0:csz[f], f, 512 * pr:512 * (pr + 1)]
                                else:
                                    rhs = xs[sect][0:csz[f], f, :]
                                nc.tensor.matmul(
                                    ps[0:mi, :],
                                    wu_k[0:csz[f], sect * F + f, 128 * mo:128 * mo + mi],
                                    rhs, start=(i == 0), stop=(i == nmm - 1))
                                i += 1
                        usl = uT[0:mi, mo, 512 * pr:512 * (pr + 1)]
                        nc.scalar.activation(usl, ps[0:mi, :],
                                             AF.Copy, bias=0.0,
                                             accum_out=uaccS[0:mi, mo, pr:pr + 1])
                        usq = sqp.tile([128, 1024], BF, tag="msq")
                        nc.scalar.activation(usq[0:mi, 0:512], usl, AF.Square,
                                             accum_out=uaccQ[0:mi, mo, pr:pr + 1])

                if DBG and k == 0:
                    nc.sync.dma_start(p_dbg_a[:], a_sb[:].rearrange("p t c -> p (t c)"))
                    nc.sync.dma_start(p_dbg_u[:], uT[:].rearrange("p m n -> p (m n)"))
                    nc.sync.dma_start(p_dbg_s[:], S[:, 0, :, :].rearrange("p h e -> p (h e)"))
                if STAGE < 6:
                    continue
                # ---- BN stats all-reduce, fold into mixing ----
                ccs = stat.tile([128, 6], F32, tag="ccs")
                nc.vector.tensor_reduce(ccs[:, 0:Fo], uaccS[:, 0:Fo, :], AX.X, AL.add)
                nc.vector.tensor_reduce(ccs[:, Fo:2 * Fo], uaccQ[:, 0:Fo, :], AX.X, AL.add)
                nc.sync.dma_start(cc_in[k][:], ccs[:, 0:2 * Fo])
                import os as _os
                if _os.environ.get("KERN_LOCAL_CC"):
                    nc.sync.dma_start(cc_out[k][:], cc_in[k][:])
                else:
                    _rg = [[i] for i in range(NCORES)] if _os.environ.get("KERN_NO_CC") else [list(range(NCORES))]
                    nc.gpsimd.collective_compute(
                        "AllReduce", AL.add, replica_groups=_rg,
                        ins=[cc_in[k].opt()], outs=[cc_out[k].opt()])
                NFILL = int(_os.environ.get("KERN_FILL", "48"))
                if NFILL and not _os.environ.get("KERN_LOCAL_CC"):
                    fill = psAgg.tile([128, 1024], F32, tag="psAgg")
                    for _fi in range(NFILL):
                        nc.tensor.matmul(fill[:, 0:512], onesbf[0:1, :],
                                         esagg[0:1, 0:512], start=True, stop=True)
                ccr = stat.tile([128, 6], F32, tag="ccr")
                nc.sync.dma_start(ccr[:, 0:2 * Fo], cc_out[k][:])
                mu = stat.tile([128, 3], F32, tag="mu")
                sc = stat.tile([128, 3], F32, tag="sc")
                mu2 = stat.tile([128, 3], F32, tag="mu2")
                _nn = float(NC if _os.environ.get("KERN_LOCAL_CC") else N)
                nc.scalar.activation(mu[:, 0:Fo], ccr[:, 0:Fo], AF.Copy, bias=0.0, scale=1.0 / _nn)
                nc.scalar.activation(sc[:, 0:Fo], ccr[:, Fo:2 * Fo], AF.Copy, bias=0.0, scale=1.0 / _nn)
                nc.vector.tensor_tensor(mu2[:, 0:Fo], mu[:, 0:Fo], mu[:, 0:Fo], AL.mult)
                nc.vector.tensor_tensor(sc[:, 0:Fo], sc[:, 0:Fo], mu2[:, 0:Fo], AL.subtract)
                nc.scalar.activation(sc[:, 0:Fo], sc[:, 0:Fo], AF.Sqrt, bias=c5[:, 0:1])
                nc.vector.reciprocal(sc[:, 0:Fo], sc[:, 0:Fo])
                nc.vector.tensor_tensor(sc[:, 0:Fo], sc[:, 0:Fo], gam[k][:, 0:Fo], AL.mult)
                for mo in range(Fo):
                    mi = msz[mo]
                    nc.gpsimd.tensor_scalar(uT[0:mi, mo, :], uT[0:mi, mo, :],
                                            mu[0:mi, mo:mo + 1], None, AL.subtract)
                    nc.vector.tensor_scalar(wxs[:, mo, 0:cout], wx[k][:, mo, 0:cout],
                                            sc[:, mo:mo + 1], None, AL.mult)
                if DBG and k == 0:
                    nc.sync.dma_start(p_dbg_bn[:, 0:6], ccs[:, 0:6])
                    nc.sync.dma_start(p_dbg_bn[:, 6:12], ccr[:, 0:6])
                    nc.sync.dma_start(p_dbg_bn[:, 12:15], mu[:, 0:3])
                    nc.sync.dma_start(p_dbg_bn[:, 15:18], sc[:, 0:3])
                    nc.sync.dma_start(p_dbg_bn[:, 18:22], uaccS[:, 0, :])
                    nc.sync.dma_start(p_dbg_bn[:, 22:26], uaccQ[:, 0, :])
                if k == 2:
                    nc.vector.memset(uT[64:65, 1, :], 1.0)
                # mixing matmul (+ BN shift via bias / ones-row), relu(leaky) = relu
                if k < 2:
                    hn = hT[k + 1]
                    for mo in range(Fo):
                        for n4 in range(4):
                            ps = psE.tile([128, 512], F32, tag="psE")
                            for mk in range(Fo):
                                nc.tensor.matmul(ps[:, :],
                                                 wxs[0:msz[mk], mk, 128 * mo:128 * (mo + 1)],
                                                 uT[0:msz[mk], mk, 512 * n4:512 * (n4 + 1)],
                                                 start=(mk == 0), stop=(mk == Fo - 1))
                            nc.scalar.activation(hn[:, mo, 512 * n4:512 * (n4 + 1)], ps[:, :],
                                                 AF.Relu, bias=bh[k][:, mo:mo + 1])
                else:
                    for t in range(16):
                        ps = psE.tile([128, 512], F32, tag="psE")
                        nc.tensor.matmul(ps[:, 0:192], uT[0:128, 0, 128 * t:128 * (t + 1)],
                                         wxs[0:128, 0, 0:192], start=True, stop=False)
                        nc.tensor.matmul(ps[:, 0:192], uT[0:65, 1, 128 * t:128 * (t + 1)],
                                         wxs[0:65, 1, 0:192], start=False, stop=True)
                        if _os.environ.get("KERN_SIM_RELU"):  # CoreSim lacks Lrelu
                            nc.scalar.activation(h3[:, t, :], ps[:, 0:192], AF.Relu)
                        else:
                            nc.scalar.activation(h3[:, t, :], ps[:, 0:192], AF.Lrelu, alpha=0.01)

            if DBG and STAGE >= 6:
                nc.sync.dma_start(p_dbg_h1[:], hT[1][:].rearrange("p m n -> p (m n)"))
                nc.sync.dma_start(p_dbg_h2[:], hT[2][:].rearrange("p m n -> p (m n)"))
                nc.sync.dma_start(p_dbg_h3[:], h3[:].rearrange("p c t -> p (c t)"))

            # ---- head --------------------------------------------------------
            if STAGE < 7:
                dummy = sall.tile([128, 16, 192], F32, tag="Sall")
                nc.vector.memset(dummy[:], 0.0)
                nc.sync.dma_start(p_out[:], dummy[:].rearrange("p c t -> p (c t)"))
            if STAGE >= 7:
                maskT = stat.tile([128, 16, 192], BF, tag="maskT")
                nc.sync.dma_start(maskT[:].rearrange("p c t -> p (c t)"), p_mask[:])
                nmx = stat.tile([128, 16], BF, tag="nmx")
                nc.vector.tensor_reduce(nmx[:], h3[:], AX.X, AL.max)
                ps3 = psE.tile([128, 512], F32, tag="psE")
                nc.tensor.matmul(ps3[0:64, 0:8], w3[:, 0, :], nmx[:, 0::2], start=True, stop=False)
                nc.tensor.matmul(ps3[0:64, 0:8], w3[:, 1, :], nmx[:, 1::2], start=False, stop=True)
                r3 = stat.tile([64, 8], BF, tag="r3")
                nc.scalar.activation(r3[:], ps3[0:64, 0:8], AF.Relu, bias=b3[:, 0:1])
                gn = stat.tile([128, 16], F32, tag="gn")
                for half in range(2):
                    ps4 = psE.tile([128, 512], F32, tag="psE")
                    nc.tensor.matmul(ps4[:, 0:8], w4[0:64, 128 * half:128 * (half + 1)], r3[:],
                                     start=True, stop=True)
                    nc.scalar.activation(gn[:, half::2], ps4[:, 0:8], AF.Sigmoid,
                                         bias=b4[:, half:half + 1])
                # feat = exp(g*h3) * mask ; per-graph softmax without max-sub
                feat = sall.tile([128, 16, 192], F32, tag="Sall")  # S dead by now
                for c in range(16):
                    nc.vector.tensor_scalar(feat[:, c, :], h3[:, c, :], gn[:, c:c + 1], None, AL.mult)
                nc.scalar.activation(feat[:], feat[:], AF.Exp)
                nc.vector.tensor_tensor(feat[:], feat[:], maskT[:], AL.mult)
                gsum = stat.tile([128, 8], F32, tag="gsum")
                nc.vector.tensor_reduce(gsum[:], feat[:].rearrange("p (g x) t -> p g (x t)", g=8), AX.X, AL.add)
                psg = psE.tile([128, 512], F32, tag="psE")
                nc.tensor.matmul(psg[0:1, 0:8], honesc[:, 0:1], gsum[:, :], start=True, stop=True)
                ginv1 = stat.tile([1, 8], F32, tag="ginv1")
                nc.vector.reciprocal(ginv1[:], psg[0:1, 0:8])
                psg2 = psE.tile([128, 512], F32, tag="psE")
                nc.tensor.matmul(psg2[:, 0:8], honesr[0:1, :], ginv1[0:1, :], start=True, stop=True)
                ginv = stat.tile([128, 8], F32, tag="ginv")
                nc.vector.tensor_copy(ginv[:], psg2[:, 0:8])
                if DBG:
                    nc.sync.dma_start(p_dbg_nmx[:], nmx[:])
                    nc.sync.dma_start(p_dbg_hd[:, 0:16], gn[:])
                    nc.sync.dma_start(p_dbg_hd[:, 16:24], gsum[:])
                    nc.sync.dma_start(p_dbg_hd[:, 24:32], ginv[:])
                for c in range(16):
                    nc.vector.tensor_scalar(feat[:, c, :], feat[:, c, :],
                                            ginv[:, c // 2:c // 2 + 1], None, AL.mult)
                nc.sync.dma_start(p_out[:], feat[:].rearrange("p c t -> p (c t)"))

    nc.compile()
    return nc


# ---------------------------------------------------------------------------
# host prep + launch
# ---------------------------------------------------------------------------

def prepare_in_maps(inputs):
    src = np.asarray(inputs["src"], np.int64)
    dst = np.asarray(inputs["dst"], np.int64)
    assert np.array_equal(dst, np.repeat(np.arange(N, dtype=np.int64), DEG)), "dst structure"
    assert np.array_equal(src // NN, dst // NN), "edges must be graph-local"

    ns = _f32(inputs["ns"]); es = _f32(inputs["es"]); dm = _f32(inputs["dm"])
    mask_fv = _f32(inputs["mask_fv"])

    Wm = [_f32(inputs[f"Wm{k + 1}"]) for k in range(3)]
    Wu = [_f32(inputs[f"Wu{k + 1}"]) for k in range(3)]
    Wx = [_f32(inputs[f"Wx{k + 1}"]) for k in range(3)]
    bx = [_f32(inputs[f"bx{k + 1}"]) for k in range(3)]
    bng = [_f32(inputs[f"bng{k + 1}"]) for k in range(3)]
    bnb = [_f32(inputs[f"bnb{k + 1}"]) for k in range(3)]

    wma_u, wmc_u, wu_u, wx_u, gam_u, bh_u = [], [], [], [], [], []
    for k in range(3):
        cin, cout, Fk, cinp, Fo = CIN[k], COUT[k], NF[k], CINP[k], NFO[k]
        Wma, Wmb, Wmce = Wm[k][:cin], Wm[k][cin:2 * cin], Wm[k][2 * cin:]
        Wmean = Wu[k][cin:2 * cin] + 8.0 * Wu[k][3 * cin:4 * cin]
        Wmax = Wu[k][2 * cin:3 * cin]
        Wstd = Wu[k][4 * cin:]
        Wh = Wu[k][:cin] + Wmb @ (Wmean + Wmax)
        a = np.zeros((128, Fk, cinp), np.float32)
        for ki in range(Fk):
            a[0:CSZ[k][ki], ki, :cin] = Wma[128 * ki:128 * ki + CSZ[k][ki]]
        wma_u.append(_bf(a.reshape(128, -1)))
        c = np.zeros((16, cinp), np.float32)
        c[:, :cin] = Wmce
        wmc_u.append(_bf(c))
        u = np.zeros((128, 4 * Fk, cout), np.float32)
        # device mean input is sum/sqrt(8) = mean*sqrt(8): pre-scale section
        for si, Wsec in enumerate([Wh, Wmean * RSQ8, Wmax, Wstd]):
            for f in range(Fk):
                u[0:CSZ[k][f], si * Fk + f, :] = Wsec[128 * f:128 * f + CSZ[k][f]]
        wu_u.append(_bf(u.reshape(128, -1)))
        if k < 2:
            x = np.zeros((128, Fo, cout), np.float32)
            gcol = np.zeros((128, Fo), np.float32)
            bcol = np.zeros((128, Fo), np.float32)
            bhv = bnb[k] @ Wx[k] + bx[k]
            for mk in range(Fo):
                m = MSZ[k][mk]
                x[0:m, mk, :] = Wx[k][128 * mk:128 * mk + m]
                gcol[0:m, mk] = bng[k][128 * mk:128 * mk + m]
                bcol[0:m, mk] = bhv[128 * mk:128 * mk + m]
            wx_u.append(_bf(x.reshape(128, -1)))
            gam_u.append(_f32(gcol))
            bh_u.append(_f32(bcol))
        else:
            x = np.zeros((128, 2, cout), np.float32)
            x[0:128, 0, :] = Wx[k][0:128]
            x[0:64, 1, :] = Wx[k][128:192]
            x[64, 1, :] = bnb[k] @ Wx[k] + bx[k]       # bias row (pairs with u ones-row)
            wx_u.append(_bf(x.reshape(128, -1)))
            gcol = np.zeros((128, 2), np.float32)
            gcol[0:128, 0] = bng[k][0:128]
            gcol[0:64, 1] = bng[k][128:192]
            gcol[64, 1] = np.sqrt(np.float32(1e-5))    # scale row becomes exactly 1.0
            gam_u.append(_f32(gcol))

    W12 = _f32(inputs["W1"]) @ _f32(inputs["W2"])
    b12v = _f32(inputs["b1"]) @ _f32(inputs["W2"]) + _f32(inputs["b2"])
    w12_u = _bf(W12.reshape(2, 128, 32).transpose(1, 0, 2).reshape(128, -1))
    w3_u = _bf(_f32(inputs["W3"]).reshape(2, 128, 64).transpose(1, 0, 2).reshape(128, -1))
    w4_u = _bf(inputs["W4"])
    b4_u = _f32(np.asarray(inputs["b4"]).reshape(2, 128).T)

    iota2 = np.stack([np.arange(128, dtype=np.float32),
                      np.arange(128, dtype=np.float32) + 128.0], axis=1)

    shared = {
        **{f"wma{k}": wma_u[k] for k in range(3)},
        **{f"wmc{k}": wmc_u[k] for k in range(3)},
        **{f"wu{k}": wu_u[k] for k in range(3)},
        **{f"wx{k}": wx_u[k] for k in range(3)},
        **{f"gam{k}": gam_u[k] for k in range(3)},
        **{f"bh{k}": bh_u[k] for k in range(2)},
        "w12": w12_u, "b12": _f32(b12v.reshape(32, 1)),
        "w3": w3_u, "b3": _f32(np.asarray(inputs["b3"]).reshape(64, 1)),
        "w4": w4_u, "b4": b4_u,
        "iota2": _f32(iota2),
        "onesbf": _bf(np.ones((1, 128), np.float32)),
        "honesc": _f32(np.ones((128, 1), np.float32)),
        "honesr": _f32(np.ones((1, 128), np.float32)),
    }

    in_maps = []
    for c in range(NCORES):
        n0 = NC * c
        e0 = 8 * n0
        esl = es[e0:e0 + EC]
        srcl = src[e0:e0 + EC] - (n0 + 256 * (np.arange(EC) // 2048))
        assert srcl.min() >= 0 and srcl.max() < 256
        madj = np.zeros((G, 256, 256), np.float32)
        for g in range(G):
            sg = src[8 * (n0 + 256 * g):8 * (n0 + 256 * (g + 1))] - (n0 + 256 * g)
            dg = dst[8 * (n0 + 256 * g):8 * (n0 + 256 * (g + 1))] - (n0 + 256 * g)
            np.add.at(madj[g], (sg, dg), 1.0)
        in_maps.append({
            "nsT": _bf(ns[n0:n0 + NC].T),
            "dmT": _bf(dm[n0:n0 + NC].T.reshape(2, 128, 2048).transpose(1, 0, 2).reshape(128, -1)),
            "esT": _bf(esl.T),
            "esagg": _bf(esl.reshape(NC, DEG, IN_E).sum(1).T),
            "srcbf": _bf(srcl.reshape(1, EC)),
            "madj": _bf(madj.reshape(G, 2, 128, 256).transpose(2, 0, 1, 3).reshape(128, -1)),
            "maskT": _bf(mask_fv[n0:n0 + NC].reshape(16, 128, 192).transpose(1, 0, 2)
                         .reshape(128, -1)),
            **shared,
        })

    return in_maps


def collect_out(res):
    out = np.zeros((B, NN * TP), np.float32)
    for c in range(NCORES):
        oc = res.results[c]["out"].reshape(128, 16, 192).transpose(1, 0, 2).reshape(NC, TP)
        out[G * c:G * (c + 1)] = oc.reshape(G, NN * TP)
    return out


def kernel(**inputs):
    in_maps = prepare_in_maps(inputs)
    nc = _BUILT.get("nc")
    if nc is None:
        nc = build_nc()
        _BUILT["nc"] = nc
    res = run_bass_kernel_spmd(nc, in_maps, list(range(NCORES)))
    _BUILT["last_results"] = res
    return collect_out(res)
